# revision 51
# baseline (speedup 1.0000x reference)
"""Trainium2 Bass kernel for nn_Condition9Trans (ConditionalTransform MLP + SO(3)
Gram-Schmidt frame with forward-mode derivative log-det).

Strategy (pure data parallel, batch N sharded across 8 NeuronCores):
  per core (padded to 65536 rows):
    mm1:  hT = relu(W1^T-free  @ featT + b1)   on PE (bf16), hT stored m-tile-major
    mm2:  mat = hT-chunks^T @ W2 + (b2 + I)    on PE (bf16 in, f32 out)
    chain: algebraically collapsed Gram-Schmidt + logdet on DVE/ACT planes:
       u,v,w = cols of rm = mat @ rot  (per-element 3x3, planar SoA layout)
       t0 = u/|u|; raw1 = v - (u.v/u.u) u; t1 = raw1/|raw1|; t2 = (u x v)/(|u||raw1|)
       logdet = 2 ln|det rm| - 2 ln(u.u) - ln(raw1.raw1)
    (identity verified numerically against the reference chain)
"""

import os
import sys
import numpy as np

sys.path.insert(0, "/opt/trn_rl_repo")

import ml_dtypes

import concourse.bass as bass
import concourse.mybir as mybir
import concourse.tile as tile
from concourse import bacc
from concourse.bass import ts, ds
from concourse.bass_utils import run_bass_kernel_spmd

F32 = mybir.dt.float32
BF16 = mybir.dt.bfloat16
AF = mybir.ActivationFunctionType
ALU = mybir.AluOpType

NCORES = 8
N_FULL = 500000
PER = N_FULL // NCORES          # 62500
NPAD = 65536                    # padded per-core rows
FD = 256
HID = 256
CH = 4096                       # elements per mm chunk
NCH = NPAD // CH                # 16 mm chunks per core
# mm chunks per chain group: small first group so DVE chain work starts early,
# small last group so the pipeline tail (last chain) is short
GROUP_SIZES = (2, 3, 4, 5, 2)
assert sum(GROUP_SIZES) == NCH
MAXG = max(GROUP_SIZES)
NG = len(GROUP_SIZES)
GROUP_STARTS = tuple(sum(GROUP_SIZES[:i]) for i in range(NG))


def _group_of_chunk(c):
    for g in range(NG):
        if c < GROUP_STARTS[g] + GROUP_SIZES[g]:
            return g, c - GROUP_STARTS[g]
    raise ValueError(c)
JW = 32                         # j (m-tile id) values per mm chunk
MMW = int(os.environ.get("K_MMW", "512"))       # mm1 free window (1 PSUM bank)
HP_BUFS = int(os.environ.get("K_HP_BUFS", "6"))
MATPS_BUFS = int(os.environ.get("K_MATPS_BUFS", "2"))
HSB_BUFS = int(os.environ.get("K_HSB_BUFS", "2"))
FEAT_BUFS = int(os.environ.get("K_FEAT_BUFS", "3"))
N_DVE_RELU = int(os.environ.get("K_NDVE", "-1"))  # -1 = use default schedule
REPS = int(os.environ.get("K_REPS", "1"))         # bench: repeat whole kernel
# relu (w, m) slots routed to DVE instead of ACT, per mm-chunk (8 slots).
# DVE-heavy early (its chain work hasn't started), ACT-only late (DVE owns
# the pipeline tail).
def _relu_dve_slots(c):
    nslots = 2 * (CH // MMW)
    if N_DVE_RELU >= 0:
        nd = N_DVE_RELU
    else:
        nd = 6 if c < 2 else 4
    if c >= NCH - 2:
        nd = 0
    step = max(1, nslots // max(1, nd)) if nd else 0
    return frozenset(range(1, nslots, step)[:nd]) if nd else frozenset()
HOST_TRANSPOSE = True           # ship featT [256, NPAD] from host (contiguous DMA)

_ONE_ACT_SET = "natural_log_exp_and_others"


def _pin_act_tables():
    """Restrict the act-table chooser to one set containing every function we
    use (relu/ln/exp/identity/abs), so the ACT engine never reloads tables."""
    import concourse.hw_specs as hw_specs
    import concourse.bacc as bacc_mod

    orig = hw_specs.get_activation_tables

    def pinned(module_arch):
        tabs = orig(module_arch)
        if _ONE_ACT_SET not in tabs:
            return tabs
        return {
            name: (funcs if name == _ONE_ACT_SET else set())
            for name, funcs in tabs.items()
        }

    bacc_mod.get_activation_tables = pinned

    # (note: walrus --enable-ldw-opt=true crashes codegen on this toolchain,
    # so redundant W1 LDWEIGHTS cannot be deduped compiler-side)


def _build_nc():
    _pin_act_tables()
    nc = bacc.Bacc()

    if HOST_TRANSPOSE:
        featbf = nc.dram_tensor("featbf", [FD, NPAD], BF16, kind="ExternalInput")
    else:
        featbf = nc.dram_tensor("featbf", [NPAD, FD], BF16, kind="ExternalInput")
    rot9 = nc.dram_tensor("rot9", [NPAD, 9], F32, kind="ExternalInput")
    w1 = nc.dram_tensor("w1", [FD, HID], BF16, kind="ExternalInput")
    w2 = nc.dram_tensor("w2", [HID, 9], BF16, kind="ExternalInput")
    b1m = nc.dram_tensor("b1m", [128, 2], F32, kind="ExternalInput")
    bconst = nc.dram_tensor("bconst", [128, JW * 9], F32, kind="ExternalInput")

    trot = nc.dram_tensor("trot", [NPAD, 9], F32, kind="ExternalOutput")
    ldet = nc.dram_tensor("ldet", [NPAD], F32, kind="ExternalOutput")

    # DRAM views for plane-mapped IO:  n = c*CH + q*JW + j  (q = partition)
    rot_v = rot9[:].rearrange("(c q j) e -> q c (j e)", q=128, j=JW)    # [128, NCH, 288]
    trot_v = trot[:].rearrange("(c q j) e -> q c (j e)", q=128, j=JW)
    ld_v = ldet[:].rearrange("(c q j) -> q c j", q=128, j=JW)           # [128, NCH, 32]

    with tile.TileContext(nc) as tc:
        with (
            tc.tile_pool(name="const", bufs=1) as constp,
            tc.tile_pool(name="feat", bufs=FEAT_BUFS) as featp,
            tc.tile_pool(name="hsb", bufs=HSB_BUFS) as hsbp,
            tc.tile_pool(name="aos", bufs=2) as aosp,
            tc.tile_pool(name="pl", bufs=2) as plp,
            tc.tile_pool(name="hpsum", bufs=HP_BUFS, space="PSUM") as hpsump,
            tc.tile_pool(name="matps", bufs=MATPS_BUFS, space="PSUM") as matpsp,
        ):
            # ---- constants ----
            w1t = constp.tile([128, 2, 2, 128], BF16, tag="w1t")   # [p, k, m, c]
            nc.sync.dma_start(
                w1t[:], w1[:].rearrange("(k p) (m c) -> p k m c", p=128, c=128)
            )
            w2t = constp.tile([128, 2, 9], BF16, tag="w2t")
            nc.sync.dma_start(w2t[:], w2[:].rearrange("(k p) e -> p k e", p=128))
            b1t = constp.tile([128, 2], F32, tag="b1t")
            nc.sync.dma_start(b1t[:], b1m[:])
            bct = constp.tile([128, JW * 9], F32, tag="bct")
            nc.sync.dma_start(bct[:], bconst[:])

            # Pre-touch constants on their consumer engines so steady-state
            # instructions don't need a second (DMA) sync wait — the 3D-AP
            # relu doesn't have encoding room for two wait commands.
            scratch = constp.tile([128, 2], F32, tag="scratch")
            nc.scalar.activation(scratch[:], b1t[:], AF.Relu)
            scratch2 = constp.tile([128, 1], F32, tag="scratch2")
            nc.vector.tensor_tensor(
                scratch2[:], bct[:, 0:1], bct[:, 1:2], ALU.add
            )

            NW = CH // MMW  # mm1 windows per chunk
            JPW = JW // NW  # mm2 m-tiles emitted per window slot

            def emit_feat_dma(c):
                feat_t = featp.tile([128, 2, CH], BF16, tag="feat")
                if HOST_TRANSPOSE:
                    src = featbf[:].rearrange("(t p) n -> p t n", p=128)
                    if c == 0:
                        # split the first load so mm1 starts sooner
                        for w in range(CH // MMW):
                            nc.sync.dma_start(
                                feat_t[:, :, ds(w * MMW, MMW)],
                                src[:, :, ds(c * CH + w * MMW, MMW)],
                            )
                    else:
                        nc.sync.dma_start(feat_t[:], src[:, :, ds(c * CH, CH)])
                elif False:
                    pass
                else:
                    for k in range(2):
                        nc.sync.dma_start_transpose(
                            feat_t[:, k, :],
                            featbf[ds(c * CH, CH), ts(k, 128)],
                        )
                return feat_t

            def emit_mm1_wgroup(feat_t, h_sb, ws, c):
                # j-sliced windows (featbf is host-permuted j-major: flat col =
                # c*CH + j*128 + q). Grouped over windows so each W1 tile is
                # loaded once per group instead of once per window.
                hp = {
                    (w, m): hpsump.tile(
                        [128, MMW], F32, tag="hpsum", name=f"hp{m}"
                    )
                    for w in ws
                    for m in range(2)
                }
                for m in range(2):
                    for k in range(2):
                        for w in ws:
                            nc.tensor.matmul(
                                hp[(w, m)][:],
                                w1t[:, k, m, :],
                                feat_t[:, k, ds(w * MMW, MMW)],
                                start=(k == 0),
                                stop=(k == 1),
                            )
                    for w in ws:
                        dst_w = h_sb[:, m, ds(w * MMW, MMW)]
                        if (w * 2 + m) % (2 * NW) in _relu_dve_slots(c):
                            nc.vector.tensor_scalar(
                                dst_w,
                                hp[(w, m)][:],
                                b1t[:, ds(m, 1)],
                                0.0,
                                ALU.add,
                                ALU.max,
                            )
                        else:
                            nc.scalar.activation(
                                dst_w,
                                hp[(w, m)][:],
                                AF.Relu,
                                bias=b1t[:, ds(m, 1)],
                            )

            def emit_mm2_slice(h_sb, mat_ps, jlo, jhi):
                for j in range(jlo, jhi):
                    for k in range(2):
                        nc.tensor.matmul(
                            mat_ps[:, ds(j * 9, 9)],
                            h_sb[:, k, ts(j, 128)],
                            w2t[:, k, :],
                            start=(k == 0),
                            stop=(k == 1),
                        )

            def plane_of(aos, e):
                return aos[:].rearrange("p c (j e) -> p (c j) e", e=9)[:, :, e]

            def alloc_group(g):
                sz = GROUP_SIZES[g]
                mat_aos = aosp.tile([128, MAXG, JW * 9], F32, tag="mataos", name="mat_aos")[:, :sz]
                rot_aos = aosp.tile([128, MAXG, JW * 9], F32, tag="rotaos", name="rot_aos")[:, :sz]
                tr_aos = aosp.tile([128, MAXG, JW * 9], F32, tag="traos", name="tr_aos")[:, :sz]
                ld_t = aosp.tile([128, MAXG, JW], F32, tag="ldt", name="ld_t")[:, :sz]
                nc.sync.dma_start(
                    rot_aos[:], rot_v[:, ds(GROUP_STARTS[g], sz), :]
                )
                return mat_aos, rot_aos, tr_aos, ld_t

            def emit_chain(g, tiles):
                mat_aos, rot_aos, tr_aos, ld_t = tiles
                fpl = GROUP_SIZES[g] * JW

                m_pl = [plane_of(mat_aos, e) for e in range(9)]   # mat[a][b] = m_pl[3a+b]
                r_pl = [plane_of(rot_aos, e) for e in range(9)]   # rot[b][c] = r_pl[3b+c]
                t_pl = [plane_of(tr_aos, e) for e in range(9)]    # t_rot[i][jc] = t_pl[3i+jc]

                def new(tag):
                    return plp.tile([128, MAXG * JW], F32, tag=tag, name=tag)[:, :fpl]

                def mul(out, a, b):
                    nc.vector.tensor_tensor(out, a, b, ALU.mult)

                def add(out, a, b):
                    nc.vector.tensor_tensor(out, a, b, ALU.add)

                def sub(out, a, b):
                    nc.vector.tensor_tensor(out, a, b, ALU.subtract)

                # rm columns u, v, w:  rm[a][c] = sum_b mat[a][b] * rot[b][c]
                tmp = new("tmp")
                cols = []
                for ci in range(3):
                    col = []
                    for a in range(3):
                        acc = new(f"rm{ci}{a}")
                        mul(acc[:], m_pl[3 * a + 0], r_pl[0 + ci])
                        mul(tmp[:], m_pl[3 * a + 1], r_pl[3 + ci])
                        add(acc[:], acc[:], tmp[:])
                        mul(tmp[:], m_pl[3 * a + 2], r_pl[6 + ci])
                        add(acc[:], acc[:], tmp[:])
                        col.append(acc)
                    cols.append(col)
                u, v, w = cols

                def dot(out, x, y):
                    mul(out[:], x[0][:], y[0][:])
                    mul(tmp[:], x[1][:], y[1][:])
                    add(out[:], out[:], tmp[:])
                    mul(tmp[:], x[2][:], y[2][:])
                    add(out[:], out[:], tmp[:])

                ss0 = new("ss0")
                dot(ss0, u, u)
                duv = new("duv")
                dot(duv, u, v)

                l0 = new("l0")
                nc.scalar.activation(l0[:], ss0[:], AF.Ln)
                inv0 = new("inv0")
                nc.scalar.activation(inv0[:], l0[:], AF.Exp, scale=-0.5)

                # t0 = inv0 * u  -> planes e = 3i
                for i in range(3):
                    mul(t_pl[3 * i + 0], inv0[:], u[i][:])

                # raw1 = v - (duv/ss0) * u ;  duv/ss0 = duv * inv0^2
                r_c = new("r_c")
                mul(r_c[:], inv0[:], inv0[:])
                mul(r_c[:], r_c[:], duv[:])
                raw1 = []
                for i in range(3):
                    t = new(f"raw1{i}")
                    mul(t[:], r_c[:], u[i][:])
                    sub(t[:], v[i][:], t[:])
                    raw1.append(t)

                ss1 = new("ss1")
                dot(ss1, raw1, raw1)
                l1 = new("l1")
                nc.scalar.activation(l1[:], ss1[:], AF.Ln)
                inv1 = new("inv1")
                nc.scalar.activation(inv1[:], l1[:], AF.Exp, scale=-0.5)

                for i in range(3):
                    mul(t_pl[3 * i + 1], inv1[:], raw1[i][:])

                # cuv = u x v ; t2 = inv0*inv1*cuv
                cuv = []
                for i in range(3):
                    i1, i2 = (i + 1) % 3, (i + 2) % 3
                    t = new(f"cuv{i}")
                    mul(t[:], u[i1][:], v[i2][:])
                    mul(tmp[:], u[i2][:], v[i1][:])
                    sub(t[:], t[:], tmp[:])
                    cuv.append(t)
                s01 = new("s01")
                mul(s01[:], inv0[:], inv1[:])
                for i in range(3):
                    mul(t_pl[3 * i + 2], s01[:], cuv[i][:])

                triple = new("triple")
                dot(triple, cuv, w)
                mul(triple[:], triple[:], triple[:])   # T^2 (avoids Abs)
                lt = new("lt")
                nc.scalar.activation(lt[:], triple[:], AF.Ln)  # ln(T^2) = 2 ln|T|

                # logdet = ln(T^2) - 2*l0 - l1
                ldp = ld_t[:].rearrange("p c j -> p (c j)")
                sub(tmp[:], lt[:], l1[:])
                nc.vector.scalar_tensor_tensor(
                    ldp, l0[:], -2.0, tmp[:], ALU.mult, ALU.add
                )

                nc.sync.dma_start(
                    trot_v[:, ds(GROUP_STARTS[g], GROUP_SIZES[g]), :], tr_aos[:]
                )
                nc.sync.dma_start(
                    ld_v[:, ds(GROUP_STARTS[g], GROUP_SIZES[g]), :], ld_t[:]
                )

            # Per chunk: each j-sliced mm1 window's relu completes its m-tiles,
            # so mm2 for those m-tiles follows immediately; the chain for a
            # group is emitted as soon as its last chunk's mat lands.
            groups = {}
            for _rep in range(REPS):
                for c in range(NCH):
                    g, cl = _group_of_chunk(c)
                    if cl == 0:
                        groups[g] = alloc_group(g)
                    feat_t = emit_feat_dma(c)
                    h_sb = hsbp.tile([128, 2, CH], BF16, tag="hsb", name="h_sb")
                    mat_ps = matpsp.tile(
                        [128, JW * 9], F32, tag="matps", name="mat_ps"
                    )
                    wgroups = [
                        tuple(range(s, min(s + 3, NW))) for s in range(0, NW, 3)
                    ]
                    for ws in wgroups:
                        emit_mm1_wgroup(feat_t, h_sb, ws, c)
                        emit_mm2_slice(
                            h_sb, mat_ps, ws[0] * JPW, (ws[-1] + 1) * JPW
                        )
                    nc.vector.tensor_tensor(
                        groups[g][0][:, cl, :], mat_ps[:], bct[:], ALU.add
                    )
                    if cl == GROUP_SIZES[g] - 1:
                        emit_chain(g, groups.pop(g))

    nc.finalize()
    return nc


_NC_CACHE = None


def _get_nc():
    global _NC_CACHE
    if _NC_CACHE is None:
        _NC_CACHE = _build_nc()
    return _NC_CACHE


def _prep_core(args):
    core, feature_bf, rotation, W1b, W2b, b1m, bct = args
    lo = core * PER
    if HOST_TRANSPOSE:
        # j-major permuted transpose: col index = c*CH + j*128 + q for batch
        # row n = c*CH + q*JW + j
        fb = np.zeros((FD, NPAD), dtype=ml_dtypes.bfloat16)
        ft = fb.reshape(FD, NPAD // CH, 128, JW)      # [fd, c, q, j] view of n
        src = np.zeros((NPAD, FD), dtype=ml_dtypes.bfloat16)
        src[:PER] = feature_bf[lo : lo + PER]
        # want fb[fd, c*CH + j*128 + q] = src[c*CH + q*JW + j, fd]
        fbv = fb.reshape(FD, NPAD // CH, JW, 128)     # [fd, c, j, q]
        fbv[:] = src.reshape(NPAD // CH, 128, JW, FD).transpose(3, 0, 2, 1)
        del ft
    else:
        fb = np.zeros((NPAD, FD), dtype=ml_dtypes.bfloat16)
        fb[:PER] = feature_bf[lo : lo + PER]
    rt = np.empty((NPAD, 9), dtype=np.float32)
    rt[:PER] = rotation[lo : lo + PER].reshape(PER, 9)
    rt[PER:] = np.eye(3, dtype=np.float32).reshape(9)
    return {
        "featbf": fb,
        "rot9": rt,
        "w1": W1b,
        "w2": W2b,
        "b1m": b1m,
        "bconst": bct,
    }


def kernel(rotation, feature, W1, b1, W2, b2, _trace=False, _tmpdir=None):
    rotation = np.ascontiguousarray(rotation, dtype=np.float32)
    W1b = np.ascontiguousarray(W1.astype(ml_dtypes.bfloat16))
    W2b = np.ascontiguousarray(W2.astype(ml_dtypes.bfloat16))
    b1m = np.ascontiguousarray(
        np.asarray(b1, dtype=np.float32).reshape(2, 128).T
    )
    bc9 = (np.asarray(b2, dtype=np.float32) + np.eye(3, dtype=np.float32).reshape(9))
    bct = np.ascontiguousarray(
        np.broadcast_to(np.tile(bc9, JW)[None, :], (128, JW * 9)).astype(np.float32)
    )

    from concurrent.futures import ThreadPoolExecutor

    feature_bf = feature.astype(ml_dtypes.bfloat16)
    with ThreadPoolExecutor(NCORES) as ex:
        in_maps = list(
            ex.map(
                _prep_core,
                [
                    (c, feature_bf, rotation, W1b, W2b, b1m, bct)
                    for c in range(NCORES)
                ],
            )
        )

    nc = _get_nc()
    res = run_bass_kernel_spmd(
        nc,
        in_maps,
        core_ids=list(range(NCORES)),
        trace=_trace,
        tmpdir=_tmpdir,
    )

    t_rot = np.empty((N_FULL, 3, 3), dtype=np.float32)
    logdet = np.empty((N_FULL,), dtype=np.float32)
    for c in range(NCORES):
        lo = c * PER
        t_rot[lo : lo + PER] = res.results[c]["trot"][:PER].reshape(PER, 3, 3)
        logdet[lo : lo + PER] = res.results[c]["ldet"][:PER]
    if _trace:
        kernel._last_exec_time_ns = res.exec_time_ns
        kernel._last_results = res
    return t_rot, logdet
